# revision 6
# baseline (speedup 1.0000x reference)
"""Trainium2 Bass kernel for CombineAttention (B=2, T=4096, sT=1024, C=1024, H=16, D=64).

Sharding: 8 cores = 2 batches x 4 head-groups (4 heads each).
Host pre-transposes activations/weights so every on-device matmul has its
contraction dim on partitions; the monotonic mask (query i attends keys
<= 4i+3) becomes a block-causal structure handled by suffix-restricted
matmuls plus one small static (128,32) diagonal-band mask.

Precision: fp16 everywhere (full PE rate, FWL weight loads, ~2^-11
element error so quantization noise is ~8x below bf16) except the
attention-weights path: exp(scores) can reach e^40, beyond fp16 range,
so exp and v are bf16 and the attn@v matmul runs in bf16. PSUM
accumulation is fp32 throughout; softmax needs no max-subtraction, and
a ones-column appended to v yields the softmax normalizer for free.

Per-core pipeline:
  qsT = WqT.T @ sxT          (256,1024)   q-scale folded into WqT on host
  kT  = WkT.T @ xT           (256,4096)   x streamed in 512-key slices
  v   = xT.T  @ WvT          (4096,256) + ones column
  per head: scoresT = kT_tile.T @ qsT ; exp ; mask band ; yT_aug = v_aug.T @ expT
  normalize rows by the ones-column sum; out_partial = yT.T_chunks @ WcT
Host sums the 4 head-group partials per batch.
"""

import math
from contextlib import ExitStack

import numpy as np
import ml_dtypes

import concourse.bass as bass
import concourse.tile as tile
from concourse import bacc, mybir
from concourse.bass import ts, ds

BF16 = mybir.dt.bfloat16
FP16 = mybir.dt.float16
FP32 = mybir.dt.float32

B = 2
C = 1024
T = 4096
ST = 1024
H = 16
D = 64
HO = 256          # head-group output channels per core (4 heads)
NCC = C // 128    # 8 contraction chunks
NTT = T // 128    # 32 key tiles
NKC = T // 512    # 8 key slices (projection streaming)
NQC = ST // 512   # 2 query chunks (attention)
NQT = ST // 128   # 8 query tiles (c-projection)
WARM_MMS = 24     # PE warmup burst to lift the HAM clock gate early


def build_nc(masked: bool = True):
    nc = bacc.Bacc("TRN2", target_bir_lowering=False, debug=False, num_devices=8)
    xT = nc.dram_tensor("xT", [C, T], FP16, kind="ExternalInput").ap()
    sxT = nc.dram_tensor("sxT", [C, ST], FP16, kind="ExternalInput").ap()
    wq = nc.dram_tensor("wq", [C, HO], FP16, kind="ExternalInput").ap()
    wk = nc.dram_tensor("wk", [C, HO], FP16, kind="ExternalInput").ap()
    wv = nc.dram_tensor("wv", [C, HO], FP16, kind="ExternalInput").ap()
    wc = nc.dram_tensor("wc", [HO, C], FP16, kind="ExternalInput").ap()
    maskd = nc.dram_tensor("mask", [128, 32], BF16, kind="ExternalInput").ap()
    out = nc.dram_tensor("out", [ST, C], FP32, kind="ExternalOutput").ap()

    with tile.TileContext(nc) as tc, ExitStack() as ctx:
        const = ctx.enter_context(tc.tile_pool(name="const", bufs=1))
        big = ctx.enter_context(tc.tile_pool(name="big", bufs=1))
        xsl_pool = ctx.enter_context(tc.tile_pool(name="xsl", bufs=6))
        work = ctx.enter_context(tc.tile_pool(name="work", bufs=6))
        nrm = ctx.enter_context(tc.tile_pool(name="nrm", bufs=3))
        outw = ctx.enter_context(tc.tile_pool(name="outw", bufs=3))

        wq_sb = const.tile([128, NCC, HO], FP16, tag="wq")
        wk_sb = const.tile([128, NCC, HO], FP16, tag="wk")
        wv_sb = const.tile([128, NCC, HO], FP16, tag="wv")
        wc_sb = const.tile([128, 2, C], FP16, tag="wc")
        mask_sb = const.tile([128, 32], BF16, tag="mask")
        warm_sb = const.tile([128, 512], BF16, tag="warm")

        kT_sb = big.tile([128, 2, T], FP16, tag="kT")
        qsT_sb = big.tile([128, 2, ST], FP16, tag="qsT")
        v_sb = big.tile([128, NTT, 4, 65], BF16, tag="v")
        yT_sb = big.tile([128, 2, ST], FP16, tag="yT")

        nc.vector.memset(warm_sb[:], 0.125)

        with tc.tile_pool(name="psA", bufs=2, space="PSUM") as pp, \
             tc.tile_pool(name="psS", bufs=4, space="PSUM") as scp, \
             tc.tile_pool(name="psV", bufs=2, space="PSUM") as avp:

            # ---- PE warmup: keep the HAM clock gate open through the DMA
            # prologue so the first real matmuls run at 2.4 GHz ----
            wps = pp.tile([128, 512], FP32, tag="proj", name="warmps")
            for i in range(WARM_MMS):
                nc.tensor.matmul(
                    wps[:], warm_sb[:, 0:128], warm_sb[:], start=True, stop=True
                )

            # ---- q projection (both query chunks share each stationary) ----
            sxsl = []
            for qc in range(NQC):
                sl = xsl_pool.tile([128, NCC, 512], FP16, tag="xsl", name=f"sxsl{qc}")
                sxsl.append(sl)
            for cc in range(NCC):
                nc.sync.dma_start(wq_sb[:, cc, :], wq[ts(cc, 128), :])
                for qc in range(NQC):
                    nc.sync.dma_start(
                        sxsl[qc][:, cc, :], sxT[ts(cc, 128), ts(qc, 512)]
                    )
            for cc in range(NCC):
                nc.sync.dma_start(wk_sb[:, cc, :], wk[ts(cc, 128), :])
            nc.sync.dma_start(wv_sb[:], wv.rearrange("(cc p) o -> p cc o", p=128))
            nc.sync.dma_start(mask_sb[:], maskd[:])

            for ot in range(2):
                pq = [pp.tile([128, 512], FP32, tag="proj", name=f"pq{qc}")
                      for qc in range(NQC)]
                for cc in range(NCC):
                    for qc in range(NQC):
                        nc.tensor.matmul(
                            pq[qc][:],
                            wq_sb[:, cc, ts(ot, 128)],
                            sxsl[qc][:, cc, :],
                            start=(cc == 0),
                            stop=(cc == NCC - 1),
                        )
                for qc in range(NQC):
                    nc.vector.tensor_copy(qsT_sb[:, ot, ts(qc, 512)], pq[qc][:])

            def proj_slice_pair(kc0):
                """k/v projections for key slices kc0, kc0+1 (stationary reuse)."""
                xsl = []
                for j in range(2):
                    sl = xsl_pool.tile(
                        [128, NCC, 512], FP16, tag="xsl", name=f"xsl{kc0 + j}"
                    )
                    for cc in range(NCC):
                        nc.sync.dma_start(
                            sl[:, cc, :], xT[ts(cc, 128), ts(kc0 + j, 512)]
                        )
                    xsl.append(sl)
                for ot in range(2):
                    pk = [pp.tile([128, 512], FP32, tag="proj", name=f"pk{j}")
                          for j in range(2)]
                    for cc in range(NCC):
                        for j in range(2):
                            nc.tensor.matmul(
                                pk[j][:],
                                wk_sb[:, cc, ts(ot, 128)],
                                xsl[j][:, cc, :],
                                start=(cc == 0),
                                stop=(cc == NCC - 1),
                            )
                    for j in range(2):
                        nc.vector.tensor_copy(
                            kT_sb[:, ot, ts(kc0 + j, 512)], pk[j][:]
                        )
                for j in range(2):
                    for tl in range(4):
                        tt = 4 * (kc0 + j) + tl
                        ps = pp.tile([128, 512], FP32, tag="proj", name="pv")
                        pv = ps[:, 0:256]
                        for cc in range(NCC):
                            nc.tensor.matmul(
                                pv,
                                xsl[j][:, cc, ts(tl, 128)],
                                wv_sb[:, cc, :],
                                start=(cc == 0),
                                stop=(cc == NCC - 1),
                            )
                        nc.vector.tensor_copy(
                            v_sb[:, tt, :, 0:64], pv.rearrange("p (h d) -> p h d", h=4)
                        )
                        nc.vector.memset(v_sb[:, tt, :, 64:65], 1.0)

            def attn_unit(ot, qc):
                """Attention for heads (2*ot, 2*ot+1), queries [512*qc, 512*qc+512)."""
                ntiles = 16 * (qc + 1) if masked else NTT
                avps = [
                    avp.tile([65, 512], FP32, tag="av", name=f"av{ot}{qc}{hh}")
                    for hh in range(2)
                ]
                for tt in range(ntiles):
                    r = tt - 16 * qc if masked else -1  # >= 0: diagonal-band tile
                    col0 = 32 * r if r >= 0 else 0
                    width = 512 - col0
                    for h in range(2):
                        row = ds(64 * h, 64)
                        sc = scp.tile([128, 512], FP32, tag="sc")
                        nc.tensor.matmul(
                            sc[:, 0:width],
                            kT_sb[row, ot, ts(tt, 128)],
                            qsT_sb[row, ot, ds(512 * qc + col0, width)],
                            start=True,
                            stop=True,
                        )
                        ex = work.tile([128, 512], BF16, tag="exp")
                        nc.scalar.activation(
                            ex[:, 0:width],
                            sc[:, 0:width],
                            mybir.ActivationFunctionType.Exp,
                        )
                        if r >= 0:
                            nc.vector.tensor_mul(ex[:, 0:32], ex[:, 0:32], mask_sb[:])
                        nc.tensor.matmul(
                            avps[h][:, ds(col0, width)],
                            v_sb[:, tt, 2 * ot + h, :],
                            ex[:, 0:width],
                            start=(tt == 0),
                            stop=(tt == ntiles - 1),
                        )
                # normalize: y = yT_unnorm / l  (l = ones-column row of av)
                for h in range(2):
                    # custom-DVE ops cannot read PSUM on HW: stage l via SBUF
                    lsb = nrm.tile([1, 512], FP32, tag="lsb")
                    nc.vector.tensor_copy(lsb[:], avps[h][64:65, :])
                    linv = nrm.tile([1, 512], FP32, tag="linv")
                    nc.vector.reciprocal_approx_fast(linv[:], lsb[:])
                    bc = nrm.tile([64, 512], FP32, tag="bc")
                    nc.sync.dma_start(
                        bc[:], linv[:].unsqueeze(1).broadcast_to([1, 64, 512])
                    )
                    nc.vector.tensor_mul(
                        yT_sb[ds(64 * h, 64), ot, ts(qc, 512)],
                        avps[h][0:64, :],
                        bc[:],
                    )

            def cproj():
                for kk in range(2):
                    nc.sync.dma_start(wc_sb[:, kk, :], wc[ts(kk, 128), :])
                for nt in range(NQT):
                    po = [pp.tile([128, 512], FP32, tag="proj", name=f"po{ec}")
                          for ec in range(2)]
                    for kk in range(2):
                        for ec in range(2):
                            nc.tensor.matmul(
                                po[ec][:],
                                yT_sb[:, kk, ts(nt, 128)],
                                wc_sb[:, kk, ts(ec, 512)],
                                start=(kk == 0),
                                stop=(kk == 1),
                            )
                    for ec in range(2):
                        osb = outw.tile([128, 512], FP32, tag="osb")
                        nc.vector.tensor_copy(osb[:], po[ec][:])
                        nc.sync.dma_start(out[ts(nt, 128), ts(ec, 512)], osb[:])

            if masked:
                for kc in range(0, 4, 2):
                    proj_slice_pair(kc)
                attn_unit(0, 0)
                attn_unit(1, 0)
                for kc in range(4, NKC, 2):
                    proj_slice_pair(kc)
                attn_unit(0, 1)
                attn_unit(1, 1)
            else:
                for kc in range(0, NKC, 2):
                    proj_slice_pair(kc)
                for qc in range(NQC):
                    for ot in range(2):
                        attn_unit(ot, qc)
            cproj()

    nc.compile()
    return nc


_NC_CACHE = {}


def _get_nc(masked: bool):
    if masked not in _NC_CACHE:
        _NC_CACHE[masked] = build_nc(masked)
    return _NC_CACHE[masked]


def _shard_inputs(x, sx, Wq, Wk, Wv, Wc, qm):
    f16 = np.float16
    bf = ml_dtypes.bfloat16
    t_len = x.shape[1]
    qscale = math.log(t_len) / math.sqrt(D)
    qmfull = np.tile(np.asarray(qm, np.float32), 4) * qscale  # (256,)

    tk = np.arange(128)[:, None]
    cl = np.arange(32)[None, :]
    mask = (cl >= tk // 4).astype(np.float32).astype(bf)

    in_maps = []
    for b in range(B):
        xT = np.ascontiguousarray(x[b].T).astype(f16)
        sxT = np.ascontiguousarray(sx[b].T).astype(f16)
        for hg in range(4):
            sl = slice(hg * HO, (hg + 1) * HO)
            in_maps.append(
                {
                    "xT": xT,
                    "sxT": sxT,
                    "wq": np.ascontiguousarray(
                        (Wq[sl, :] * qmfull[:, None]).T
                    ).astype(f16),
                    "wk": np.ascontiguousarray(Wk[sl, :].T).astype(f16),
                    "wv": np.ascontiguousarray(Wv[sl, :].T).astype(f16),
                    "wc": np.ascontiguousarray(Wc[:, sl].T).astype(f16),
                    "mask": mask,
                }
            )
    return in_maps


def _run(inputs, trace=False):
    from concourse.bass_utils import run_bass_kernel_spmd

    x = np.asarray(inputs["x"], np.float32)
    sx = np.asarray(inputs["sx"], np.float32)
    Wq = np.asarray(inputs["Wq"], np.float32)
    Wk = np.asarray(inputs["Wk"], np.float32)
    Wv = np.asarray(inputs["Wv"], np.float32)
    Wc = np.asarray(inputs["Wc"], np.float32)
    qm = np.asarray(inputs["qm"], np.float32)
    causal = int(np.asarray(inputs.get("causal", 1)))
    masked = bool(causal) and sx.shape[1] != x.shape[1]

    nc = _get_nc(masked)
    in_maps = _shard_inputs(x, sx, Wq, Wk, Wv, Wc, qm)
    kwargs = {}
    if trace:
        kwargs = dict(trace=True, trace_cores=list(range(8)))
    res = run_bass_kernel_spmd(nc, in_maps, core_ids=list(range(8)), **kwargs)

    out = np.zeros((B, ST, C), np.float32)
    for b in range(B):
        for hg in range(4):
            out[b] += res.results[b * 4 + hg]["out"]
    return out, res


def kernel(**inputs):
    out, _ = _run(inputs, trace=False)
    return out


def kernel_traced(**inputs):
    out, res = _run(inputs, trace=True)
    return out, res


# revision 9
# speedup vs baseline: 1.3560x; 1.3560x over previous
"""Trainium2 Bass kernel for CombineAttention (B=2, T=4096, sT=1024, C=1024, H=16, D=64).

Sharding: 8 cores = 2 batches x 4 head-groups (4 heads each).
Host pre-transposes activations/weights so every on-device matmul has its
contraction dim on partitions; the monotonic mask (query i attends keys
<= 4i+3) becomes a block-causal structure handled by suffix-restricted
matmuls plus one small static (128,32) diagonal-band mask.

Precision: fp16 everywhere (full PE rate, FWL weight loads, ~2^-11
element error so quantization noise is ~8x below bf16) except the
attention-weights path: exp(scores) can reach e^40, beyond fp16 range,
so exp and v are bf16 and the attn@v matmul runs in bf16. PSUM
accumulation is fp32 throughout; softmax needs no max-subtraction, and
a ones-column appended to v yields the softmax normalizer for free.

Per-core pipeline:
  qsT = WqT.T @ sxT          (256,1024)   q-scale folded into WqT on host
  kT  = WkT.T @ xT           (256,4096)   x streamed in 512-key slices
  v   = xT.T  @ WvT          (4096,256) + ones column
  per head: scoresT = kT_tile.T @ qsT ; exp ; mask band ; yT_aug = v_aug.T @ expT
  normalize rows by the ones-column sum; out_partial = yT.T_chunks @ WcT
Host sums the 4 head-group partials per batch.
"""

import math
from contextlib import ExitStack

import numpy as np
import ml_dtypes

import concourse.bass as bass
import concourse.tile as tile
from concourse import bacc, mybir
from concourse.bass import ts, ds

BF16 = mybir.dt.bfloat16
FP16 = mybir.dt.float16
FP32 = mybir.dt.float32

B = 2
C = 1024
T = 4096
ST = 1024
H = 16
D = 64
HO = 256          # head-group output channels per core (4 heads)
NCC = C // 128    # 8 contraction chunks
NTT = T // 128    # 32 key tiles
NKC = T // 512    # 8 key slices (projection streaming)
NQC = ST // 512   # 2 query chunks (attention)
NQT = ST // 128   # 8 query tiles (c-projection)
WARM_MMS = 24     # PE warmup burst to lift the HAM clock gate early


def build_nc(masked: bool = True):
    nc = bacc.Bacc("TRN2", target_bir_lowering=False, debug=False, num_devices=8)
    xT = nc.dram_tensor("xT", [C, T], FP16, kind="ExternalInput").ap()
    sxT = nc.dram_tensor("sxT", [C, ST], FP16, kind="ExternalInput").ap()
    wq = nc.dram_tensor("wq", [C, HO], FP16, kind="ExternalInput").ap()
    wk = nc.dram_tensor("wk", [C, HO], FP16, kind="ExternalInput").ap()
    wv = nc.dram_tensor("wv", [C, HO], FP16, kind="ExternalInput").ap()
    wc = nc.dram_tensor("wc", [HO, C], FP16, kind="ExternalInput").ap()
    maskd = nc.dram_tensor("mask", [128, 32], BF16, kind="ExternalInput").ap()
    out = nc.dram_tensor("out", [ST, C], FP32, kind="ExternalOutput").ap()

    with tile.TileContext(nc) as tc, ExitStack() as ctx:
        const = ctx.enter_context(tc.tile_pool(name="const", bufs=1))
        big = ctx.enter_context(tc.tile_pool(name="big", bufs=1))
        xsl_pool = ctx.enter_context(tc.tile_pool(name="xsl", bufs=6))
        work = ctx.enter_context(tc.tile_pool(name="work", bufs=6))
        nrm = ctx.enter_context(tc.tile_pool(name="nrm", bufs=3))
        outw = ctx.enter_context(tc.tile_pool(name="outw", bufs=3))

        wq_sb = const.tile([128, NCC, HO], FP16, tag="wq")
        wk_sb = const.tile([128, NCC, HO], FP16, tag="wk")
        wv_sb = const.tile([128, NCC, HO], FP16, tag="wv")
        wc_sb = const.tile([128, 2, C], FP16, tag="wc")
        mask_sb = const.tile([128, 32], BF16, tag="mask")
        warm_sb = const.tile([128, 512], BF16, tag="warm")

        kT_sb = big.tile([128, 2, T], FP16, tag="kT")
        qsT_sb = big.tile([128, 2, ST], FP16, tag="qsT")
        v_sb = big.tile([128, NTT, 4, 65], BF16, tag="v")
        yT_sb = big.tile([128, 2, ST], FP16, tag="yT")

        nc.vector.memset(warm_sb[:], 0.125)

        with tc.tile_pool(name="psA", bufs=2, space="PSUM") as pp, \
             tc.tile_pool(name="psS", bufs=2, space="PSUM") as scp, \
             tc.tile_pool(name="psV", bufs=2, space="PSUM") as avp:

            # ---- PE warmup: keep the HAM clock gate open through the DMA
            # prologue so the first real matmuls run at 2.4 GHz ----
            wps = pp.tile([128, 512], FP32, tag="proj", name="warmps")
            for i in range(WARM_MMS):
                nc.tensor.matmul(
                    wps[:], warm_sb[:, 0:128], warm_sb[:], start=True, stop=True
                )

            # ---- q projection (both query chunks share each stationary) ----
            sxsl = []
            for qc in range(NQC):
                sl = xsl_pool.tile([128, NCC, 512], FP16, tag="xsl", name=f"sxsl{qc}")
                sxsl.append(sl)
            for cc in range(NCC):
                nc.sync.dma_start(wq_sb[:, cc, :], wq[ts(cc, 128), :])
                for qc in range(NQC):
                    nc.sync.dma_start(
                        sxsl[qc][:, cc, :], sxT[ts(cc, 128), ts(qc, 512)]
                    )
            for cc in range(NCC):
                nc.sync.dma_start(wk_sb[:, cc, :], wk[ts(cc, 128), :])
            nc.sync.dma_start(wv_sb[:], wv.rearrange("(cc p) o -> p cc o", p=128))
            nc.sync.dma_start(mask_sb[:], maskd[:])

            for ot in range(2):
                pq = [pp.tile([128, 512], FP32, tag="proj", name=f"pq{qc}")
                      for qc in range(NQC)]
                for cc in range(NCC):
                    for qc in range(NQC):
                        nc.tensor.matmul(
                            pq[qc][:],
                            wq_sb[:, cc, ts(ot, 128)],
                            sxsl[qc][:, cc, :],
                            start=(cc == 0),
                            stop=(cc == NCC - 1),
                        )
                for qc in range(NQC):
                    nc.vector.tensor_copy(qsT_sb[:, ot, ts(qc, 512)], pq[qc][:])

            def proj_slice_pair(kc0):
                """k/v projections for key slices kc0, kc0+1 (stationary reuse)."""
                xsl = []
                for j in range(2):
                    sl = xsl_pool.tile(
                        [128, NCC, 512], FP16, tag="xsl", name=f"xsl{kc0 + j}"
                    )
                    for cc in range(NCC):
                        nc.sync.dma_start(
                            sl[:, cc, :], xT[ts(cc, 128), ts(kc0 + j, 512)]
                        )
                    xsl.append(sl)
                for ot in range(2):
                    pk = [pp.tile([128, 512], FP32, tag="proj", name=f"pk{j}")
                          for j in range(2)]
                    for cc in range(NCC):
                        for j in range(2):
                            nc.tensor.matmul(
                                pk[j][:],
                                wk_sb[:, cc, ts(ot, 128)],
                                xsl[j][:, cc, :],
                                start=(cc == 0),
                                stop=(cc == NCC - 1),
                            )
                    for j in range(2):
                        nc.vector.tensor_copy(
                            kT_sb[:, ot, ts(kc0 + j, 512)], pk[j][:]
                        )
                for j in range(2):
                    for tl in range(4):
                        tt = 4 * (kc0 + j) + tl
                        ps = pp.tile([128, 512], FP32, tag="proj", name="pv")
                        pv = ps[:, 0:256]
                        for cc in range(NCC):
                            nc.tensor.matmul(
                                pv,
                                xsl[j][:, cc, ts(tl, 128)],
                                wv_sb[:, cc, :],
                                start=(cc == 0),
                                stop=(cc == NCC - 1),
                            )
                        nc.vector.tensor_copy(
                            v_sb[:, tt, :, 0:64], pv.rearrange("p (h d) -> p h d", h=4)
                        )
                        nc.vector.memset(v_sb[:, tt, :, 64:65], 1.0)

            def attn_unit(ot, qc):
                """Attention for heads (2*ot, 2*ot+1), queries [512*qc, 512*qc+512)."""
                ntiles = 16 * (qc + 1) if masked else NTT
                avps = [
                    avp.tile([65, 512], FP32, tag="av", name=f"av{ot}{qc}{hh}")
                    for hh in range(2)
                ]
                for tt in range(ntiles):
                    r = tt - 16 * qc if masked else -1  # >= 0: diagonal-band tile
                    col0 = 32 * r if r >= 0 else 0
                    width = 512 - col0
                    # both heads' scores go into one 2-bank psum tile, h0 at
                    # the end of bank 0 and h1 at the start of bank 1, so a
                    # single gap-free ACTIVATE (352-cycle fixed cost) covers
                    # the pair
                    base = 512 - width
                    sc = scp.tile([128, 1024], FP32, tag="sc")
                    for h in range(2):
                        row = ds(64 * h, 64)
                        nc.tensor.matmul(
                            sc[:, ds(base + width * h, width)],
                            kT_sb[row, ot, ts(tt, 128)],
                            qsT_sb[row, ot, ds(512 * qc + col0, width)],
                            start=True,
                            stop=True,
                        )
                    ex = work.tile([128, 1024], BF16, tag="exp")
                    nc.scalar.activation(
                        ex[:, ds(base, 2 * width)],
                        sc[:, ds(base, 2 * width)],
                        mybir.ActivationFunctionType.Exp,
                    )
                    if r >= 0:
                        exb = ex[:, ds(base, 2 * width)].rearrange(
                            "p (g x) -> p g x", g=2
                        )[:, :, 0:32]
                        nc.vector.tensor_mul(
                            exb,
                            exb,
                            mask_sb[:].unsqueeze(1).broadcast_to([128, 2, 32]),
                        )
                    for h in range(2):
                        nc.tensor.matmul(
                            avps[h][:, ds(col0, width)],
                            v_sb[:, tt, 2 * ot + h, :],
                            ex[:, ds(base + width * h, width)],
                            start=(tt == 0),
                            stop=(tt == ntiles - 1),
                        )
                # normalize: y = yT_unnorm / l  (l = ones-column row of av)
                for h in range(2):
                    # custom-DVE ops cannot read PSUM on HW: stage l via SBUF
                    lsb = nrm.tile([1, 512], FP32, tag="lsb")
                    nc.vector.tensor_copy(lsb[:], avps[h][64:65, :])
                    linv = nrm.tile([1, 512], FP32, tag="linv")
                    nc.vector.reciprocal_approx_fast(linv[:], lsb[:])
                    bc = nrm.tile([64, 512], FP32, tag="bc")
                    nc.sync.dma_start(
                        bc[:], linv[:].unsqueeze(1).broadcast_to([1, 64, 512])
                    )
                    nc.vector.tensor_mul(
                        yT_sb[ds(64 * h, 64), ot, ts(qc, 512)],
                        avps[h][0:64, :],
                        bc[:],
                    )

            def cproj():
                for kk in range(2):
                    nc.sync.dma_start(wc_sb[:, kk, :], wc[ts(kk, 128), :])
                for nt in range(NQT):
                    po = [pp.tile([128, 512], FP32, tag="proj", name=f"po{ec}")
                          for ec in range(2)]
                    for kk in range(2):
                        for ec in range(2):
                            nc.tensor.matmul(
                                po[ec][:],
                                yT_sb[:, kk, ts(nt, 128)],
                                wc_sb[:, kk, ts(ec, 512)],
                                start=(kk == 0),
                                stop=(kk == 1),
                            )
                    for ec in range(2):
                        osb = outw.tile([128, 512], FP32, tag="osb")
                        nc.vector.tensor_copy(osb[:], po[ec][:])
                        nc.sync.dma_start(out[ts(nt, 128), ts(ec, 512)], osb[:])

            if masked:
                for kc in range(0, 4, 2):
                    proj_slice_pair(kc)
                attn_unit(0, 0)
                attn_unit(1, 0)
                for kc in range(4, NKC, 2):
                    proj_slice_pair(kc)
                attn_unit(0, 1)
                attn_unit(1, 1)
            else:
                for kc in range(0, NKC, 2):
                    proj_slice_pair(kc)
                for qc in range(NQC):
                    for ot in range(2):
                        attn_unit(ot, qc)
            cproj()

    nc.compile()
    return nc


_NC_CACHE = {}


def _get_nc(masked: bool):
    if masked not in _NC_CACHE:
        _NC_CACHE[masked] = build_nc(masked)
    return _NC_CACHE[masked]


def _shard_inputs(x, sx, Wq, Wk, Wv, Wc, qm):
    f16 = np.float16
    bf = ml_dtypes.bfloat16
    t_len = x.shape[1]
    qscale = math.log(t_len) / math.sqrt(D)
    qmfull = np.tile(np.asarray(qm, np.float32), 4) * qscale  # (256,)

    tk = np.arange(128)[:, None]
    cl = np.arange(32)[None, :]
    mask = (cl >= tk // 4).astype(np.float32).astype(bf)

    in_maps = []
    for b in range(B):
        xT = np.ascontiguousarray(x[b].T).astype(f16)
        sxT = np.ascontiguousarray(sx[b].T).astype(f16)
        for hg in range(4):
            sl = slice(hg * HO, (hg + 1) * HO)
            in_maps.append(
                {
                    "xT": xT,
                    "sxT": sxT,
                    "wq": np.ascontiguousarray(
                        (Wq[sl, :] * qmfull[:, None]).T
                    ).astype(f16),
                    "wk": np.ascontiguousarray(Wk[sl, :].T).astype(f16),
                    "wv": np.ascontiguousarray(Wv[sl, :].T).astype(f16),
                    "wc": np.ascontiguousarray(Wc[:, sl].T).astype(f16),
                    "mask": mask,
                }
            )
    return in_maps


def _run(inputs, trace=False):
    from concourse.bass_utils import run_bass_kernel_spmd

    x = np.asarray(inputs["x"], np.float32)
    sx = np.asarray(inputs["sx"], np.float32)
    Wq = np.asarray(inputs["Wq"], np.float32)
    Wk = np.asarray(inputs["Wk"], np.float32)
    Wv = np.asarray(inputs["Wv"], np.float32)
    Wc = np.asarray(inputs["Wc"], np.float32)
    qm = np.asarray(inputs["qm"], np.float32)
    causal = int(np.asarray(inputs.get("causal", 1)))
    masked = bool(causal) and sx.shape[1] != x.shape[1]

    nc = _get_nc(masked)
    in_maps = _shard_inputs(x, sx, Wq, Wk, Wv, Wc, qm)
    kwargs = {}
    if trace:
        kwargs = dict(trace=True, trace_cores=list(range(8)))
    res = run_bass_kernel_spmd(nc, in_maps, core_ids=list(range(8)), **kwargs)

    out = np.zeros((B, ST, C), np.float32)
    for b in range(B):
        for hg in range(4):
            out[b] += res.results[b * 4 + hg]["out"]
    return out, res


def kernel(**inputs):
    out, _ = _run(inputs, trace=False)
    return out


def kernel_traced(**inputs):
    out, res = _run(inputs, trace=True)
    return out, res
